# revision 1
# baseline (speedup 1.0000x reference)
"""AttentiveFP forward on 8 Trainium2 NeuronCores (Bass/Tile).

Nodes sharded across 8 cores at molecule boundaries. Edges sorted by
destination; per shard, destination nodes are degree-sorted into 50
blocks of 128 (nodes on partitions) with padded per-node edge slots in
the free dim, processed in chunks of <=24 slots. Messages arrive via
int16-indexed dma_gather (table base biased +32768 rows to cover the
51200-row replicated table); softmax+aggregation are masked multiply +
strided reduce in numerator/denominator form (no segment max: exp
ratios are shift-invariant and logits are O(1)). GRU runs H-major with
split-K matmuls. Layer-2/3 tables are exchanged via AllGather; the
molecule readout is shard-local.
"""
import sys, os, time
sys.path.insert(0, "/opt/trn_rl_repo")

import numpy as np
import types, contextlib, ctypes


def _install_hook_mod():
    if "antenv.axon_hooks" in sys.modules:
        return
    mod = types.ModuleType("antenv.axon_hooks")
    _so = "/opt/axon/libaxon_pjrt.so"

    def _build():
        if not os.path.exists(_so):
            return None
        lib = ctypes.CDLL(_so)
        if not hasattr(lib, "axon_start_nrt_profile"):
            return None
        lib.axon_start_nrt_profile.argtypes = [ctypes.POINTER(ctypes.c_int64), ctypes.c_size_t]
        lib.axon_start_nrt_profile.restype = ctypes.c_int64
        lib.axon_stop_nrt_profile.argtypes = [ctypes.c_char_p]
        lib.axon_stop_nrt_profile.restype = ctypes.c_int64

        @contextlib.contextmanager
        def _hook(output_dir, device_ids):
            import jax
            jax.devices()
            if device_ids:
                ids = (ctypes.c_int64 * len(device_ids))(*device_ids)
                rc = lib.axon_start_nrt_profile(ids, len(device_ids))
            else:
                rc = lib.axon_start_nrt_profile(None, 0)
            if rc != 0:
                raise RuntimeError(f"axon_start_nrt_profile rc={rc}")
            try:
                yield
            finally:
                n = lib.axon_stop_nrt_profile(str(output_dir).encode())
                print(f"profile: {n} file(s) -> {output_dir}", file=sys.stderr)

        return _hook

    _h = [None]

    def get_axon_ntff_profile_hook():
        if _h[0] is None:
            _h[0] = _build()
        return _h[0]

    mod.get_axon_ntff_profile_hook = get_axon_ntff_profile_hook
    mod.set_axon_ntff_profile_hook = lambda h: _h.__setitem__(0, h)
    sys.modules["antenv.axon_hooks"] = mod


_install_hook_mod()

import concourse.bass as bass
import concourse.bacc as bacc
import concourse.mybir as mybir
import concourse.tile as tile
from concourse.bass_utils import run_bass_kernel_spmd
from concourse.masks import make_identity

F32 = mybir.dt.float32
I16 = mybir.dt.int16
AF = mybir.ActivationFunctionType
OP = mybir.AluOpType

N, E, H, ED, G = 50000, 1000000, 64, 16, 512
IN_C, OUT_C, L, T, OUT_N = 39, 128, 3, 2, 1
SLOPE = 0.01
NDEV = 8
NP = 6400
NB = NP // 128
NTAB = NDEV * NP
BIAS = 32768
WC = 24          # slot-chunk width

TRACE = False
LAST_EXEC_NS = [None]


def _bap(ap, dims, extra_off=0):
    return bass.AP(ap.tensor, ap.offset + extra_off, dims)


def _wrap_flat_idx(flat):
    n = flat.shape[0]
    w16 = flat.reshape(n // 16, 16).T
    return np.tile(w16, (8, 1)).astype(np.int16)


# ======================================================================
def preprocess(inputs):
    x = np.asarray(inputs["x"], np.float32)
    ei = np.asarray(inputs["edge_index"], np.int64)
    ea = np.asarray(inputs["edge_attr"], np.float32)
    batch = np.asarray(inputs["batch"], np.int64)
    src, dst = ei[0], ei[1]

    order = np.argsort(dst, kind="stable")
    src_s, dst_s, ea_s = src[order], dst[order], ea[order]

    gb = np.searchsorted(batch, np.arange(G + 1))
    cuts, gcuts = [0], [0]
    for k in range(1, NDEV):
        tgt = k * N // NDEV
        gi = int(np.abs(gb - tgt).argmin())
        gcuts.append(gi)
        cuts.append(int(gb[gi]))
    cuts.append(N)
    gcuts.append(G)
    nk = np.diff(cuts)
    gkc = np.diff(gcuts)
    assert (nk <= NP).all(), nk
    GMAX = int(gkc.max())
    assert GMAX <= 128
    ecuts = np.searchsorted(dst_s, cuts)

    pos_k = np.zeros((NDEV, NP), np.int64)
    degs_sorted = np.zeros((NDEV, NP), np.int64)
    grow = np.zeros(N, np.int64)
    for k in range(NDEV):
        ldst = dst_s[ecuts[k]:ecuts[k + 1]] - cuts[k]
        deg = np.bincount(ldst, minlength=NP)
        sortp = np.argsort(-deg, kind="stable")
        pos = np.zeros(NP, np.int64)
        pos[sortp] = np.arange(NP)
        pos_k[k] = pos
        degs_sorted[k] = deg[sortp]
        grow[cuts[k]:cuts[k + 1]] = k * NP + pos[:nk[k]]

    W_b = []
    for b in range(NB):
        w = int(max(1, degs_sorted[:, b * 128].max()))
        if degs_sorted[:, b * 128 + 127].max() == w:
            w += 1
        W_b.append(w)
    Wmax = max(W_b)
    sumW = sum(W_b)
    cumW = np.concatenate([[0], np.cumsum(W_b)]).astype(np.int64)

    DUM16 = np.int16(NTAB - 1 - BIAS)
    sidx = np.full((NDEV, NB, 128, Wmax), DUM16, np.int16)
    mask = np.zeros((NDEV, NB, 128, Wmax), np.float32)
    easl = np.zeros((NDEV, NB, ED, Wmax * 128), np.float32)
    for k in range(NDEV):
        e0, e1 = ecuts[k], ecuts[k + 1]
        ldst = dst_s[e0:e1] - cuts[k]
        q = pos_k[k][ldst]
        blk = q // 128
        prt = q % 128
        grp_start = np.searchsorted(ldst, np.arange(NP))
        w_e = np.arange(e1 - e0) - grp_start[ldst]
        rows = (grow[src_s[e0:e1]] - BIAS).astype(np.int16)
        sidx[k, blk, prt, w_e] = rows
        mask[k, blk, prt, w_e] = 1.0
        col = w_e * 128 + prt
        eat = ea_s[e0:e1]
        for a in range(ED):
            easl[k, blk, a, col] = eat[:, a]

    idx_w = np.zeros((NDEV, NB, 128, 8 * Wmax), np.int16)
    mask_f = np.zeros((NDEV, 128, sumW), np.float32)
    for k in range(NDEV):
        for b in range(NB):
            w = W_b[b]
            flat = sidx[k, b, :, :w].T.reshape(-1)
            idx_w[k, b, :, :8 * w] = _wrap_flat_idx(flat)
            mask_f[k, :, cumW[b]:cumW[b + 1]] = mask[k, b, :, :w]

    gsizes = np.diff(gb)
    Wg = int(gsizes.max()) + 1
    sidxm = np.full((NDEV, 128, Wg), np.int16(NP - 1), np.int16)
    maskm = np.zeros((NDEV, 128, Wg), np.float32)
    for k in range(NDEV):
        j = np.arange(nk[k])
        gl = batch[cuts[k]:cuts[k + 1]] - gcuts[k]
        w = (cuts[k] + j) - gb[gcuts[k] + gl]
        sidxm[k, gl, w] = pos_k[k][j].astype(np.int16)
        maskm[k, gl, w] = 1.0
    idxm_w = np.zeros((NDEV, 128, 8 * Wg), np.int16)
    for k in range(NDEV):
        idxm_w[k] = _wrap_flat_idx(sidxm[k].T.reshape(-1))

    xg = np.zeros((IN_C, NTAB), np.float32)
    xg[:, grow] = x.T
    xo = np.zeros((NDEV, IN_C, NP), np.float32)
    for k in range(NDEV):
        xo[k] = xg[:, k * NP:(k + 1) * NP]

    meta = dict(W_b=W_b, Wmax=Wmax, sumW=sumW, cumW=cumW, Wg=Wg, GMAX=GMAX,
                cuts=cuts, gcuts=gcuts, nk=nk, gkc=gkc)
    arrs = dict(xg=xg, xo=xo, idx_w=idx_w, mask_f=mask_f, easl=easl,
                idxm_w=idxm_w, maskm=maskm)
    return meta, arrs


def pack_weights(inputs):
    g = lambda n: np.asarray(inputs[n], np.float32)
    wd = {}
    wd["lin1_lhsT"] = g("lin1_w").T.copy()
    wd["b1col"] = g("lin1_b")[:, None].copy()
    A = g("g_lin1_w")[:, :H]
    B = g("g_lin1_w")[:, H:]
    wd["rhs_u"] = A.T.copy()
    wd["rhs_w2"] = g("g_lin2_w").T.copy()
    wd["BT"] = B.T.copy()
    wd["gl_rep"] = np.tile(g("g_att_l")[None, :], (128, 1)).copy()
    wd["gar_col"] = g("g_att_r")[:, None].copy()
    wd["gbias_col"] = g("g_bias")[:, None].copy()

    def gru_pack(pfx, wi, wh, bi, bh):
        bi = bi - wi.sum(1)
        wd[pfx + "wi_r"] = wi[0:H].T.copy()
        wd[pfx + "wh_r"] = wh[0:H].T.copy()
        wd[pfx + "wi_z"] = wi[H:2 * H].T.copy()
        wd[pfx + "wh_z"] = wh[H:2 * H].T.copy()
        wd[pfx + "wi_n"] = wi[2 * H:].T.copy()
        wd[pfx + "wh_n"] = wh[2 * H:].T.copy()
        wd[pfx + "br"] = (bi[0:H] + bh[0:H])[:, None].copy()
        wd[pfx + "bz"] = (bi[H:2 * H] + bh[H:2 * H])[:, None].copy()
        wd[pfx + "bin"] = bi[2 * H:][:, None].copy()
        wd[pfx + "bhn"] = bh[2 * H:][:, None].copy()

    gru_pack("g0_", g("gru0_wi"), g("gru0_wh"), g("gru0_bi"), g("gru0_bh"))
    for l in range(L - 1):
        wd[f"c{l}_rhs"] = g("conv_lin_w")[l].T.copy()
        wd[f"c{l}_asrep"] = np.tile(g("conv_att_src")[l][None, :], (128, 1)).copy()
        wd[f"c{l}_wad"] = (g("conv_lin_w")[l].T @ g("conv_att_dst")[l])[:, None].copy()
        wd[f"c{l}_bias"] = g("conv_bias")[l][:, None].copy()
        gru_pack(f"c{l}_", g("grul_wi")[l], g("grul_wh")[l],
                 g("grul_bi")[l], g("grul_bh")[l])
    wd["rhs_mol"] = np.concatenate([np.eye(H, dtype=np.float32),
                                    g("mol_lin_w").T], 1).copy()
    wd["m_asrep"] = np.tile(g("mol_att_src")[None, :], (128, 1)).copy()
    wd["m_wad"] = (g("mol_lin_w").T @ g("mol_att_dst"))[:, None].copy()
    wd["m_bias"] = g("mol_bias")[:, None].copy()
    gru_pack("m_", g("mgru_wi"), g("mgru_wh"), g("mgru_bi"), g("mgru_bh"))
    wd["lin2_lhsT"] = g("lin2_w").T.copy()
    wd["lin2_bcol"] = g("lin2_b")[:, None].copy()
    return wd


# ======================================================================
def build_kernel(meta, wshapes):
    W_b, Wmax, sumW = meta["W_b"], meta["Wmax"], meta["sumW"]
    cumW, Wg = meta["cumW"], meta["Wg"]
    nc = bacc.Bacc(None, num_swdge_queues=4)

    dp = lambda n, s, dt=F32: nc.declare_dram_parameter(n, list(s), dt, isOutput=False)
    xg_d = dp("xg", [IN_C, NTAB])
    xo_d = dp("xo", [IN_C, NP])
    idx_d = dp("idx_w", [NB, 128, 8 * Wmax], I16)
    maskf_d = dp("mask_f", [128, sumW])
    easl_d = dp("easl", [NB, ED, Wmax * 128])
    idxm_d = dp("idxm_w", [128, 8 * Wg], I16)
    maskm_d = dp("maskm", [128, Wg])
    w_d = {n: dp(n, s) for n, s in wshapes.items()}
    out_d = nc.declare_dram_parameter("out", [OUT_C, 128], F32, isOutput=True)

    qn = [0]

    def nextq():
        qn[0] = (qn[0] + 1) % 4
        return qn[0]

    with tile.TileContext(nc) as tc:
        with tc.tile_pool(name="const", bufs=1) as cp, \
             tc.tile_pool(name="state", bufs=1) as st, \
             tc.tile_pool(name="wk5", bufs=6) as w5, \
             tc.tile_pool(name="wkio", bufs=3) as wio, \
             tc.tile_pool(name="ztp", bufs=3) as ztp, \
             tc.tile_pool(name="gbuf", bufs=2) as gp, \
             tc.tile_pool(name="small", bufs=2) as sm, \
             tc.tile_pool(name="gps", bufs=4, space="PSUM") as psg, \
             tc.tile_pool(name="vps", bufs=2, space="PSUM") as psv, \
             tc.tile_pool(name="mps", bufs=2, space="PSUM") as psm, \
             tc.tile_pool(name="dram", bufs=1, space="DRAM") as dr:

            wt = {}
            for n in wshapes:
                t = cp.tile(list(wshapes[n]), F32, tag=n)
                nc.sync.dma_start(t[:], w_d[n][:])
                wt[n] = t
            ident = cp.tile([128, 128], F32, tag="ident")
            make_identity(nc, ident[:])
            maskf = cp.tile([128, sumW], F32, tag="maskf")
            nc.sync.dma_start(maskf[:], maskf_d[:])
            maskm = cp.tile([128, Wg], F32, tag="maskm")
            nc.sync.dma_start(maskm[:], maskm_d[:])
            rcol = cp.tile([128, NB], F32, tag="rcol")
            adcol = cp.tile([128, NB], F32, tag="adcol")
            asrm = cp.tile([128, Wg], F32, tag="asrm")
            outT = cp.tile([H, 128], F32, tag="outT")

            xA = st.tile([H, NP], F32, tag="xA")
            xB = st.tile([H, NP], F32, tag="xB")
            hT = st.tile([H, NP], F32, tag="hT")

            table1 = dr.tile([NTAB, 2 * H], F32, tag="t1")
            tb_in = dr.tile([NP, H], F32, tag="tbin")
            table2 = dr.tile([NTAB, H], F32, tag="t2", addr_space="Shared")
            table3 = dr.tile([NTAB, H], F32, tag="t3", addr_space="Shared")
            molt = dr.tile([NP, 2 * H], F32, tag="molt")

            t1_biased = _bap(table1[:], [[2 * H, NTAB - BIAS], [1, 2 * H]],
                             BIAS * 2 * H)

            def gather_chunk(buf, tab_ap, b, w0, cw, elem):
                it = sm.tile([128, 8 * WC], I16, tag="idx")
                nc.sync.dma_start(it[:, :8 * cw], idx_d[b, :, 8 * w0:8 * (w0 + cw)])
                nc.gpsimd.dma_gather(
                    out_ap=buf[:, 0:cw, :], in_ap=tab_ap, idxs_ap=it[:, 0:8 * cw],
                    num_idxs=128 * cw, num_idxs_reg=128 * cw,
                    elem_size=elem, single_packet=False, queue_num=nextq())

            def softmax_chunk(asr, mask_ap, adcol_ap, cw):
                cnd = sm.tile([128, WC], F32, tag="cnd")
                nc.vector.tensor_scalar(out=cnd[:, :cw], in0=asr, scalar1=adcol_ap,
                                        scalar2=None, op0=OP.add)
                nc.scalar.activation(cnd[:, :cw], cnd[:, :cw], AF.Lrelu, alpha=SLOPE)
                nc.scalar.activation(cnd[:, :cw], cnd[:, :cw], AF.Exp)
                pm = sm.tile([128, WC], F32, tag="pm")
                nc.vector.tensor_tensor(out=pm[:, :cw], in0=cnd[:, :cw],
                                        in1=mask_ap, op=OP.mult)
                return pm

            def agg_chunk(msg_ap, pm, cw, rstride, num, zz, first):
                pmb = _bap(pm[:, 0:1], [pm[:].ap[0], [1, cw], [0, H]])
                nc.vector.tensor_tensor(out=msg_ap, in0=msg_ap, in1=pmb, op=OP.mult)
                mr = _bap(msg_ap, [msg_ap.ap[0], [1, H], [rstride, cw]])
                if first:
                    nc.vector.tensor_reduce(out=num[:], in_=mr,
                                            axis=mybir.AxisListType.X, op=OP.add)
                    nc.vector.tensor_reduce(out=zz[:], in_=pm[:, 0:cw],
                                            axis=mybir.AxisListType.X, op=OP.add)
                else:
                    part = sm.tile([128, H], F32, tag="part")
                    nc.vector.tensor_reduce(out=part[:], in_=mr,
                                            axis=mybir.AxisListType.X, op=OP.add)
                    nc.vector.tensor_tensor(out=num[:], in0=num[:], in1=part[:],
                                            op=OP.add)
                    zp = sm.tile([128, 1], F32, tag="zp")
                    nc.vector.tensor_reduce(out=zp[:], in_=pm[:, 0:cw],
                                            axis=mybir.AxisListType.X, op=OP.add)
                    nc.vector.tensor_tensor(out=zz[:], in0=zz[:], in1=zp[:],
                                            op=OP.add)

            def finish_block(num, zz, bias_col, b):
                nc.vector.tensor_scalar(out=zz[:], in0=zz[:], scalar1=1e-16,
                                        scalar2=None, op0=OP.add)
                rec = sm.tile([128, 1], F32, tag="rec")
                nc.vector.reciprocal(rec[:], zz[:])
                hnm = sm.tile([128, H], F32, tag="hnm")
                nc.vector.tensor_scalar(out=hnm[:], in0=num[:], scalar1=rec[:, 0:1],
                                        scalar2=None, op0=OP.mult)
                tps = psv.tile([H, 128], F32, tag="vps")
                nc.tensor.transpose(out=tps[:], in_=hnm[:], identity=ident[:])
                rp = sm.tile([H, 128], F32, tag="rp")
                nc.scalar.activation(rp[:], tps[:], AF.Relu, bias=bias_col)
                m0 = sm.tile([H, 128], F32, tag="m0")
                nc.vector.tensor_scalar(out=m0[:], in0=tps[:], scalar1=bias_col,
                                        scalar2=None, op0=OP.add)
                nc.vector.tensor_scalar(out=m0[:], in0=m0[:], scalar1=0.0,
                                        scalar2=None, op0=OP.min)
                nc.scalar.activation(m0[:], m0[:], AF.Exp)
                nc.vector.tensor_tensor(out=hT[:, b * 128:(b + 1) * 128],
                                        in0=rp[:], in1=m0[:], op=OP.add)

            def gru(pfx, h_ap_full, x_in, x_out, nloc):
                c0 = 0
                while c0 < nloc:
                    cw = min(512, nloc - c0)
                    sl = slice(c0, c0 + cw)
                    h_ap = h_ap_full[:, sl]
                    x_ap = x_in[:, sl]

                    rps = psg.tile([H, 512], F32, tag="gp")
                    nc.tensor.matmul(rps[:, :cw], lhsT=wt[pfx + "wi_r"][:],
                                     rhs=h_ap, start=True, stop=False)
                    nc.tensor.matmul(rps[:, :cw], lhsT=wt[pfx + "wh_r"][:],
                                     rhs=x_ap, start=False, stop=True)
                    rsb = w5.tile([H, 512], F32, tag="w5")
                    nc.scalar.activation(rsb[:, :cw], rps[:, :cw], AF.Sigmoid,
                                         bias=wt[pfx + "br"][:])
                    zps = psg.tile([H, 512], F32, tag="gp")
                    nc.tensor.matmul(zps[:, :cw], lhsT=wt[pfx + "wi_z"][:],
                                     rhs=h_ap, start=True, stop=False)
                    nc.tensor.matmul(zps[:, :cw], lhsT=wt[pfx + "wh_z"][:],
                                     rhs=x_ap, start=False, stop=True)
                    zsb = w5.tile([H, 512], F32, tag="w5")
                    nc.scalar.activation(zsb[:, :cw], zps[:, :cw], AF.Sigmoid,
                                         bias=wt[pfx + "bz"][:])
                    gin = psg.tile([H, 512], F32, tag="gp")
                    nc.tensor.matmul(gin[:, :cw], lhsT=wt[pfx + "wi_n"][:],
                                     rhs=h_ap, start=True, stop=True)
                    ghn = psg.tile([H, 512], F32, tag="gp")
                    nc.tensor.matmul(ghn[:, :cw], lhsT=wt[pfx + "wh_n"][:],
                                     rhs=x_ap, start=True, stop=True)
                    ghb = w5.tile([H, 512], F32, tag="w5")
                    nc.scalar.activation(ghb[:, :cw], ghn[:, :cw], AF.Identity,
                                         bias=wt[pfx + "bhn"][:])
                    rg = w5.tile([H, 512], F32, tag="w5")
                    nc.vector.tensor_tensor(out=rg[:, :cw], in0=rsb[:, :cw],
                                            in1=ghb[:, :cw], op=OP.mult)
                    nc.vector.tensor_tensor(out=rg[:, :cw], in0=rg[:, :cw],
                                            in1=gin[:, :cw], op=OP.add)
                    nsb = w5.tile([H, 512], F32, tag="w5")
                    nc.scalar.activation(nsb[:, :cw], rg[:, :cw], AF.Tanh,
                                         bias=wt[pfx + "bin"][:])
                    dd = w5.tile([H, 512], F32, tag="w5")
                    nc.vector.tensor_tensor(out=dd[:, :cw], in0=x_ap,
                                            in1=nsb[:, :cw], op=OP.subtract)
                    nc.vector.tensor_tensor(out=dd[:, :cw], in0=dd[:, :cw],
                                            in1=zsb[:, :cw], op=OP.mult)
                    nc.vector.tensor_tensor(out=dd[:, :cw], in0=dd[:, :cw],
                                            in1=nsb[:, :cw], op=OP.add)
                    nc.scalar.activation(x_out[:, sl], dd[:, :cw], AF.Relu)
                    c0 += cw

            # ---------- stage 0 ----------
            for c in range(NTAB // 512):
                xc = wio.tile([IN_C, 512], F32, tag="wkio")
                nc.sync.dma_start(xc[:], xg_d[:, c * 512:(c + 1) * 512])
                x1p = psg.tile([H, 512], F32, tag="gp")
                nc.tensor.matmul(x1p[:], lhsT=wt["lin1_lhsT"][:], rhs=xc[:],
                                 start=True, stop=True)
                x1s = wio.tile([H, 512], F32, tag="wkio")
                nc.scalar.activation(x1s[:], x1p[:], AF.Lrelu,
                                     bias=wt["b1col"][:], alpha=SLOPE)
                for s in range(4):
                    tps = psm.tile([128, 128], F32, tag="mps")
                    nc.tensor.matmul(tps[:, 0:H], lhsT=x1s[:, s * 128:(s + 1) * 128],
                                     rhs=wt["rhs_u"][:], start=True, stop=True)
                    nc.tensor.matmul(tps[:, H:2 * H], lhsT=x1s[:, s * 128:(s + 1) * 128],
                                     rhs=wt["rhs_w2"][:], start=True, stop=True)
                    tsb = w5.tile([128, 128], F32, tag="w5")
                    nc.vector.tensor_copy(tsb[:], tps[:])
                    nc.sync.dma_start(
                        table1[c * 512 + s * 128: c * 512 + (s + 1) * 128, :], tsb[:])

            c0 = 0
            while c0 < NP:
                cw = min(512, NP - c0)
                xc = wio.tile([IN_C, 512], F32, tag="wkio")
                nc.sync.dma_start(xc[:, :cw], xo_d[:, c0:c0 + cw])
                x1p = psg.tile([H, 512], F32, tag="gp")
                nc.tensor.matmul(x1p[:, :cw], lhsT=wt["lin1_lhsT"][:], rhs=xc[:, :cw],
                                 start=True, stop=True)
                nc.scalar.activation(xA[:, c0:c0 + cw], x1p[:, :cw], AF.Lrelu,
                                     bias=wt["b1col"][:], alpha=SLOPE)
                c0 += cw
            for b in range(NB):
                rp_ = psm.tile([128, 128], F32, tag="mps")
                nc.tensor.matmul(rp_[:, 0:1], lhsT=xA[:, b * 128:(b + 1) * 128],
                                 rhs=wt["gar_col"][:], start=True, stop=True)
                nc.vector.tensor_copy(rcol[:, b:b + 1], rp_[:, 0:1])

            # ---------- layer 1: GATEConv ----------
            for b in range(NB):
                w = W_b[b]
                num = sm.tile([128, H], F32, tag="num")
                zz = sm.tile([128, 1], F32, tag="zz")
                w0 = 0
                first = True
                while w0 < w:
                    cw = min(WC, w - w0)
                    bufE = gp.tile([128, WC, 2 * H], F32, tag="gbuf")
                    gather_chunk(bufE, t1_biased, b, w0, cw, 2 * H)
                    at = sm.tile([128, WC], F32, tag="at")
                    for g0 in range(0, cw, 8):
                        gw = min(8, cw - g0)
                        eat = wio.tile([ED, 8 * 128], F32, tag="wkio")
                        nc.sync.dma_start(
                            eat[:, :gw * 128],
                            easl_d[b, :, (w0 + g0) * 128:(w0 + g0 + gw) * 128])
                        vps = psv.tile([128, 8, H], F32, tag="vps")
                        for j in range(gw):
                            nc.tensor.matmul(vps[:, j, :],
                                             lhsT=eat[:, j * 128:(j + 1) * 128],
                                             rhs=wt["BT"][:], start=True, stop=True)
                        zt = ztp.tile([128, 8, H], F32, tag="zt")
                        nc.vector.tensor_tensor(out=zt[:, :gw, :],
                                                in0=bufE[:, g0:g0 + gw, 0:H],
                                                in1=vps[:, :gw, :], op=OP.add)
                        nc.scalar.activation(zt[:, :gw, :], zt[:, :gw, :], AF.Lrelu,
                                             alpha=SLOPE)
                        glb = _bap(wt["gl_rep"][:],
                                   [wt["gl_rep"][:].ap[0], [0, gw], [1, H]])
                        nc.vector.tensor_tensor(out=zt[:, :gw, :], in0=zt[:, :gw, :],
                                                in1=glb, op=OP.mult)
                        nc.vector.tensor_reduce(out=at[:, g0:g0 + gw],
                                                in_=zt[:, :gw, :],
                                                axis=mybir.AxisListType.X, op=OP.add)
                    pm = softmax_chunk(at[:, 0:cw],
                                       maskf[:, cumW[b] + w0:cumW[b] + w0 + cw],
                                       rcol[:, b:b + 1], cw)
                    agg_chunk(bufE[:, 0:cw, H:2 * H], pm, cw, 2 * H, num, zz, first)
                    first = False
                    w0 += cw
                finish_block(num, zz, wt["gbias_col"][:], b)
            gru("g0_", hT, xA, xB, NP)

            # ---------- GATConv layers ----------
            def build_table(x_src, rhs_t, wad_t, tbl_rows, cdim):
                for b in range(NB):
                    tps = psm.tile([128, 128], F32, tag="mps")
                    nc.tensor.matmul(tps[:, 0:cdim],
                                     lhsT=x_src[:, b * 128:(b + 1) * 128],
                                     rhs=rhs_t, start=True, stop=True)
                    if wad_t is not None:
                        nc.tensor.matmul(tps[:, cdim:cdim + 1],
                                         lhsT=x_src[:, b * 128:(b + 1) * 128],
                                         rhs=wad_t, start=True, stop=True)
                        nc.vector.tensor_copy(adcol[:, b:b + 1],
                                              tps[:, cdim:cdim + 1])
                    tsb = w5.tile([128, 128], F32, tag="w5")
                    nc.vector.tensor_copy(tsb[:, :cdim], tps[:, 0:cdim])
                    nc.sync.dma_start(tbl_rows[b * 128:(b + 1) * 128, :],
                                      tsb[:, :cdim])

            def gatconv(lidx, x_in, x_out, table_full):
                build_table(x_in, wt[f"c{lidx}_rhs"][:], wt[f"c{lidx}_wad"][:],
                            tb_in, H)
                nc.gpsimd.collective_compute(
                    "AllGather", OP.bypass, replica_groups=[list(range(NDEV))],
                    ins=[tb_in[:]], outs=[table_full[:]])
                tbias = _bap(table_full[:], [[H, NTAB - BIAS], [1, H]], BIAS * H)
                for b in range(NB):
                    w = W_b[b]
                    num = sm.tile([128, H], F32, tag="num")
                    zz = sm.tile([128, 1], F32, tag="zz")
                    w0 = 0
                    first = True
                    while w0 < w:
                        cw = min(WC, w - w0)
                        buf = gp.tile([128, WC, H], F32, tag="gbuf")
                        gather_chunk(buf, tbias, b, w0, cw, H)
                        at = sm.tile([128, WC], F32, tag="at")
                        for g0 in range(0, cw, 8):
                            gw = min(8, cw - g0)
                            zt = ztp.tile([128, 8, H], F32, tag="zt")
                            asb = _bap(wt[f"c{lidx}_asrep"][:],
                                       [wt[f"c{lidx}_asrep"][:].ap[0], [0, gw], [1, H]])
                            nc.vector.tensor_tensor(out=zt[:, :gw, :],
                                                    in0=buf[:, g0:g0 + gw, :],
                                                    in1=asb, op=OP.mult)
                            nc.vector.tensor_reduce(out=at[:, g0:g0 + gw],
                                                    in_=zt[:, :gw, :],
                                                    axis=mybir.AxisListType.X,
                                                    op=OP.add)
                        pm = softmax_chunk(at[:, 0:cw],
                                           maskf[:, cumW[b] + w0:cumW[b] + w0 + cw],
                                           adcol[:, b:b + 1], cw)
                        agg_chunk(buf[:, 0:cw, :], pm, cw, H, num, zz, first)
                        first = False
                        w0 += cw
                    finish_block(num, zz, wt[f"c{lidx}_bias"][:], b)
                gru(f"c{lidx}_", hT, x_in, x_out, NP)

            gatconv(0, xB, xA, table2)
            gatconv(1, xA, xB, table3)
            x4 = xB

            # ---------- molecule readout ----------
            build_table(x4, wt["rhs_mol"][:], None, molt, 2 * H)

            NCH = (Wg + WC - 1) // WC

            def mol_gather(w0, cw):
                bufm = gp.tile([128, WC, 2 * H], F32, tag="gbuf")
                it = sm.tile([128, 8 * WC], I16, tag="idx")
                nc.sync.dma_start(it[:, :8 * cw], idxm_d[:, 8 * w0:8 * (w0 + cw)])
                nc.gpsimd.dma_gather(
                    out_ap=bufm[:, 0:cw, :], in_ap=molt[:], idxs_ap=it[:, 0:8 * cw],
                    num_idxs=128 * cw, num_idxs_reg=128 * cw,
                    elem_size=2 * H, single_packet=False, queue_num=nextq())
                return bufm

            sumx = sm.tile([128, H], F32, tag="sumx")
            nc.vector.memset(sumx[:], 0.0)
            for ci in range(NCH):
                w0 = ci * WC
                cw = min(WC, Wg - w0)
                bufm = mol_gather(w0, cw)
                for g0 in range(0, cw, 8):
                    gw = min(8, cw - g0)
                    zt = ztp.tile([128, 8, H], F32, tag="zt")
                    mb = _bap(maskm[:, w0 + g0:w0 + g0 + 1],
                              [maskm[:].ap[0], [1, gw], [0, H]])
                    nc.vector.tensor_tensor(out=zt[:, :gw, :],
                                            in0=bufm[:, g0:g0 + gw, 0:H],
                                            in1=mb, op=OP.mult)
                    part = sm.tile([128, H], F32, tag="part")
                    mr = _bap(zt[:, 0:gw, :], [zt[:].ap[0], [1, H], [H, gw]])
                    nc.vector.tensor_reduce(out=part[:], in_=mr,
                                            axis=mybir.AxisListType.X, op=OP.add)
                    nc.vector.tensor_tensor(out=sumx[:], in0=sumx[:], in1=part[:],
                                            op=OP.add)
                    arb = _bap(wt["m_asrep"][:],
                               [wt["m_asrep"][:].ap[0], [0, gw], [1, H]])
                    nc.vector.tensor_tensor(out=zt[:, :gw, :],
                                            in0=bufm[:, g0:g0 + gw, H:2 * H],
                                            in1=arb, op=OP.mult)
                    nc.vector.tensor_reduce(out=asrm[:, w0 + g0:w0 + g0 + gw],
                                            in_=zt[:, :gw, :],
                                            axis=mybir.AxisListType.X, op=OP.add)
            o0ps = psv.tile([H, 128], F32, tag="vps")
            nc.tensor.transpose(out=o0ps[:], in_=sumx[:], identity=ident[:])
            nc.scalar.activation(outT[:], o0ps[:], AF.Relu)

            hm = cp.tile([H, 128], F32, tag="hm")
            gout = cp.tile([H, 128], F32, tag="gout")
            for t in range(T):
                adp = psm.tile([128, 128], F32, tag="mps")
                nc.tensor.matmul(adp[:, 0:1], lhsT=outT[:], rhs=wt["m_wad"][:],
                                 start=True, stop=True)
                adc = sm.tile([128, 1], F32, tag="adc")
                nc.vector.tensor_copy(adc[:], adp[:, 0:1])
                cndm = sm.tile([128, Wg], F32, tag="cndm")
                nc.vector.tensor_scalar(out=cndm[:], in0=asrm[:], scalar1=adc[:, 0:1],
                                        scalar2=None, op0=OP.add)
                nc.scalar.activation(cndm[:], cndm[:], AF.Lrelu, alpha=SLOPE)
                nc.scalar.activation(cndm[:], cndm[:], AF.Exp)
                pmm = sm.tile([128, Wg], F32, tag="pmm")
                nc.vector.tensor_tensor(out=pmm[:], in0=cndm[:], in1=maskm[:],
                                        op=OP.mult)
                num = sm.tile([128, H], F32, tag="num")
                zz = sm.tile([128, 1], F32, tag="zz")
                first = True
                for ci in range(NCH):
                    w0 = ci * WC
                    cw = min(WC, Wg - w0)
                    bufm = mol_gather(w0, cw)
                    agg_chunk(bufm[:, 0:cw, H:2 * H], pmm[:, w0:w0 + cw], cw,
                              2 * H, num, zz, first)
                    first = False
                nc.vector.tensor_scalar(out=zz[:], in0=zz[:], scalar1=1e-16,
                                        scalar2=None, op0=OP.add)
                rec = sm.tile([128, 1], F32, tag="rec")
                nc.vector.reciprocal(rec[:], zz[:])
                hnm = sm.tile([128, H], F32, tag="hnm")
                nc.vector.tensor_scalar(out=hnm[:], in0=num[:], scalar1=rec[:, 0:1],
                                        scalar2=None, op0=OP.mult)
                tps = psv.tile([H, 128], F32, tag="vps")
                nc.tensor.transpose(out=tps[:], in_=hnm[:], identity=ident[:])
                rp = sm.tile([H, 128], F32, tag="rp")
                nc.scalar.activation(rp[:], tps[:], AF.Relu, bias=wt["m_bias"][:])
                m0 = sm.tile([H, 128], F32, tag="m0")
                nc.vector.tensor_scalar(out=m0[:], in0=tps[:], scalar1=wt["m_bias"][:],
                                        scalar2=None, op0=OP.add)
                nc.vector.tensor_scalar(out=m0[:], in0=m0[:], scalar1=0.0,
                                        scalar2=None, op0=OP.min)
                nc.scalar.activation(m0[:], m0[:], AF.Exp)
                nc.vector.tensor_tensor(out=hm[:], in0=rp[:], in1=m0[:], op=OP.add)
                gru("m_", hm, outT, gout, 128)
                nc.vector.tensor_copy(outT[:], gout[:])

            resp = psm.tile([128, 128], F32, tag="mps")
            nc.tensor.matmul(resp[:], lhsT=wt["lin2_lhsT"][:], rhs=outT[:],
                             start=True, stop=True)
            rsb = sm.tile([OUT_C, 128], F32, tag="rsb2")
            nc.scalar.activation(rsb[:], resp[:], AF.Identity, bias=wt["lin2_bcol"][:])
            nc.sync.dma_start(out_d[:], rsb[:])

    nc.compile()
    return nc


# ======================================================================
def kernel(**inputs):
    meta, arrs = preprocess(inputs)
    wd = pack_weights(inputs)
    wshapes = {n: v.shape for n, v in wd.items()}
    t0 = time.time()
    nc = build_kernel(meta, wshapes)
    print(f"[kernel] build+compile {time.time()-t0:.1f}s", file=sys.stderr)

    in_maps = []
    for k in range(NDEV):
        m = dict(xg=arrs["xg"], xo=arrs["xo"][k], idx_w=arrs["idx_w"][k],
                 mask_f=arrs["mask_f"][k], easl=arrs["easl"][k],
                 idxm_w=arrs["idxm_w"][k], maskm=arrs["maskm"][k])
        m.update(wd)
        in_maps.append(m)

    res = run_bass_kernel_spmd(nc, in_maps, list(range(NDEV)), trace=TRACE)
    LAST_EXEC_NS[0] = res.exec_time_ns

    out = np.zeros((G, OUT_C), np.float32)
    gcuts = meta["gcuts"]
    for k in range(NDEV):
        gk = meta["gkc"][k]
        out[gcuts[k]:gcuts[k] + gk] = res.results[k]["out"][:, :gk].T
    return np.repeat(out[None], OUT_N, axis=0)



# revision 33
# speedup vs baseline: 1.3417x; 1.3417x over previous
"""AttentiveFP forward on 8 Trainium2 NeuronCores (Bass/Tile).

Nodes sharded across 8 cores at molecule boundaries; edges sorted by
destination with degree-sorted 128-node dst blocks and padded per-node
edge slots (as in the original layout). Optimizations over the v1
kernel: gather tables are bf16 [NTAB, 128] rows (256B elements) so all
per-edge vector math runs at 2x DVE rate; edge indices are preloaded
once and shared by all three conv layers so the Q7 gather stream runs
back-to-back; layer-1 edge_attr projections are batched 8 slots per
matmul via a block-diagonal B^T; the molecule readout is matmul-ized
with host-built segment one-hots (no gathers at all); stage0 builds the
layer-1 table H-major with XBAR DMA transposes; GRU + next-layer table
build are emitted inside the block loop so they overlap the gather
stream, leaving only the AllGather serial between layers.
"""
import sys, os, time
sys.path.insert(0, "/opt/trn_rl_repo")

import numpy as np
import types, contextlib, ctypes
import ml_dtypes

BFNP = ml_dtypes.bfloat16


def _install_hook_mod():
    if "antenv.axon_hooks" in sys.modules:
        return
    mod = types.ModuleType("antenv.axon_hooks")
    _so = "/opt/axon/libaxon_pjrt.so"

    def _build():
        if not os.path.exists(_so):
            return None
        lib = ctypes.CDLL(_so)
        if not hasattr(lib, "axon_start_nrt_profile"):
            return None
        lib.axon_start_nrt_profile.argtypes = [ctypes.POINTER(ctypes.c_int64), ctypes.c_size_t]
        lib.axon_start_nrt_profile.restype = ctypes.c_int64
        lib.axon_stop_nrt_profile.argtypes = [ctypes.c_char_p]
        lib.axon_stop_nrt_profile.restype = ctypes.c_int64

        @contextlib.contextmanager
        def _hook(output_dir, device_ids):
            import jax
            jax.devices()
            if device_ids:
                ids = (ctypes.c_int64 * len(device_ids))(*device_ids)
                rc = lib.axon_start_nrt_profile(ids, len(device_ids))
            else:
                rc = lib.axon_start_nrt_profile(None, 0)
            if rc != 0:
                raise RuntimeError(f"axon_start_nrt_profile rc={rc}")
            try:
                yield
            finally:
                n = lib.axon_stop_nrt_profile(str(output_dir).encode())
                print(f"profile: {n} file(s) -> {output_dir}", file=sys.stderr)

        return _hook

    _h = [None]

    def get_axon_ntff_profile_hook():
        if _h[0] is None:
            _h[0] = _build()
        return _h[0]

    mod.get_axon_ntff_profile_hook = get_axon_ntff_profile_hook
    mod.set_axon_ntff_profile_hook = lambda h: _h.__setitem__(0, h)
    sys.modules["antenv.axon_hooks"] = mod


_install_hook_mod()

import concourse.bass as bass
import concourse.bacc as bacc
import concourse.mybir as mybir
import concourse.tile as tile
from concourse.bass_utils import run_bass_kernel_spmd
from concourse.masks import make_identity

F32 = mybir.dt.float32
BF16 = mybir.dt.bfloat16
I16 = mybir.dt.int16
AF = mybir.ActivationFunctionType
OP = mybir.AluOpType

N, E, H, ED, G = 50000, 1000000, 64, 16, 512
IN_C, OUT_C, L, T, OUT_N = 39, 128, 3, 2, 1
SLOPE = 0.01
NDEV = 8
NP = 6400
NB = NP // 128
NTAB = NDEV * NP
BIAS = 32768
WC = 24          # slot-chunk width
GMAX = 66        # max molecules per core (upper bound, checked in preprocess)

TRACE = False
DEBUG = False
ONEQ = False
LAST_EXEC_NS = [None]
LAST_RESULTS = [None]


def _bap(ap, dims, extra_off=0):
    return bass.AP(ap.tensor, ap.offset + extra_off, dims)


def _wrap_flat_idx(flat):
    n = flat.shape[0]
    w16 = flat.reshape(n // 16, 16).T
    return np.tile(w16, (8, 1)).astype(np.int16)


# ======================================================================
def preprocess(inputs):
    x = np.asarray(inputs["x"], np.float32)
    ei = np.asarray(inputs["edge_index"], np.int64)
    ea = np.asarray(inputs["edge_attr"], np.float32)
    batch = np.asarray(inputs["batch"], np.int64)
    src, dst = ei[0], ei[1]

    order = np.argsort(dst, kind="stable")
    src_s, dst_s, ea_s = src[order], dst[order], ea[order]

    gb = np.searchsorted(batch, np.arange(G + 1))
    cuts, gcuts = [0], [0]
    for k in range(1, NDEV):
        tgt = k * N // NDEV
        gi = int(np.abs(gb - tgt).argmin())
        gcuts.append(gi)
        cuts.append(int(gb[gi]))
    cuts.append(N)
    gcuts.append(G)
    nk = np.diff(cuts)
    gkc = np.diff(gcuts)
    assert (nk <= NP).all(), nk
    assert int(gkc.max()) <= GMAX
    ecuts = np.searchsorted(dst_s, cuts)

    pos_k = np.zeros((NDEV, NP), np.int64)
    degs_sorted = np.zeros((NDEV, NP), np.int64)
    grow = np.zeros(N, np.int64)
    for k in range(NDEV):
        ldst = dst_s[ecuts[k]:ecuts[k + 1]] - cuts[k]
        deg = np.bincount(ldst, minlength=NP)
        sortp = np.argsort(-deg, kind="stable")
        pos = np.zeros(NP, np.int64)
        pos[sortp] = np.arange(NP)
        pos_k[k] = pos
        degs_sorted[k] = deg[sortp]
        grow[cuts[k]:cuts[k + 1]] = k * NP + pos[:nk[k]]

    W_b = []
    for b in range(NB):
        w = int(max(1, degs_sorted[:, b * 128].max()))
        if degs_sorted[:, b * 128 + 127].max() == w:
            w += 1
        W_b.append(w)
    sumW = sum(W_b)
    cumW = np.concatenate([[0], np.cumsum(W_b)]).astype(np.int64)
    ngrp = [(w + 7) // 8 for w in W_b]
    goff = np.concatenate([[0], np.cumsum(ngrp)]).astype(np.int64)
    Gtot = int(goff[-1])

    DUM16 = np.int16(NTAB - 1 - BIAS)
    Wmax = max(W_b)
    sidx = np.full((NDEV, NB, 128, Wmax), DUM16, np.int16)
    mask = np.zeros((NDEV, NB, 128, Wmax), np.float32)
    easl = np.zeros((NDEV, Gtot, 128, 128), np.float32)
    for k in range(NDEV):
        e0, e1 = ecuts[k], ecuts[k + 1]
        ldst = dst_s[e0:e1] - cuts[k]
        q = pos_k[k][ldst]
        blk = q // 128
        prt = q % 128
        grp_start = np.searchsorted(ldst, np.arange(NP))
        w_e = np.arange(e1 - e0) - grp_start[ldst]
        rows = (grow[src_s[e0:e1]] - BIAS).astype(np.int16)
        sidx[k, blk, prt, w_e] = rows
        mask[k, blk, prt, w_e] = 1.0
        g_e = goff[blk] + w_e // 8
        j_e = (w_e % 8) * 16
        eat = ea_s[e0:e1]
        easl[k, g_e[:, None], j_e[:, None] + np.arange(ED)[None, :], prt[:, None]] = eat

    # per-chunk idx columns with one trailing all-dummy slot per chunk
    # (the dma_gather completion can race its last few elements; the pad
    # slot absorbs the race and is never consumed)
    chunks = []          # (b, w0, cw, chunk_col_offset)
    coff = 0
    for b in range(NB):
        w0 = 0
        while w0 < W_b[b]:
            cw = min(WC, W_b[b] - w0)
            chunks.append((b, w0, cw, coff))
            coff += cw + 1
            w0 += cw
    totslots = coff
    idx_all = np.full((NDEV, 128, 8 * totslots), DUM16, np.int16)
    mask_f = np.zeros((NDEV, 128, sumW), np.float32)
    for k in range(NDEV):
        for (b, w0, cw, co) in chunks:
            flat = sidx[k, b, :, w0:w0 + cw].T.reshape(-1)
            idx_all[k, :, 8 * co:8 * (co + cw)] = _wrap_flat_idx(flat)
        for b in range(NB):
            mask_f[k, :, cumW[b]:cumW[b + 1]] = mask[k, b, :, :W_b[b]]

    # molecule one-hot inputs: local mol id per (partition, block), -1 pad
    molcol = np.full((NDEV, 128, NB), -1.0, np.float32)
    for k in range(NDEV):
        inv = np.zeros(NP, np.int64)
        inv[pos_k[k][:nk[k]]] = np.arange(nk[k])
        valid = np.zeros(NP, bool)
        valid[pos_k[k][:nk[k]]] = True
        gl = batch[cuts[k] + inv] - gcuts[k]
        mc = np.where(valid, gl.astype(np.float32), -1.0)
        molcol[k] = mc.reshape(NB, 128).T
    iotaF = np.tile(np.arange(GMAX, dtype=np.float32)[None, :], (128, 1))

    xg16 = np.zeros((IN_C, NTAB), np.float32)
    xg16[:, grow] = x.T
    xo = np.zeros((NDEV, IN_C, NP), np.float32)
    for k in range(NDEV):
        xo[k] = xg16[:, k * NP:(k + 1) * NP]

    meta = dict(W_b=W_b, sumW=sumW, cumW=cumW, ngrp=ngrp, goff=goff, Gtot=Gtot,
                cuts=cuts, gcuts=gcuts, nk=nk, gkc=gkc, pos_k=pos_k, grow=grow,
                ecuts=ecuts, order=order, chunks=chunks, totslots=totslots)
    arrs = dict(xg16=xg16.astype(BFNP), xo=xo,
                idx_all=idx_all, mask_f=mask_f,
                easl=easl.astype(BFNP), molcol=molcol, iotaF=iotaF)
    return meta, arrs


def pack_weights(inputs):
    g = lambda n: np.asarray(inputs[n], np.float32)
    wd = {}
    wd["lin1_lhsT"] = g("lin1_w").T.copy()                      # f32 [39,64]
    wd["lin1_lhsT16"] = g("lin1_w").T.astype(BFNP).copy()
    wd["b1col"] = g("lin1_b")[:, None].copy()
    A = g("g_lin1_w")[:, :H]
    B = g("g_lin1_w")[:, H:]
    wd["m1_lhsT"] = np.hstack([A.T, g("g_lin2_w").T]).astype(BFNP).copy()  # [64,128]
    # block-diagonal B^T: [8*ED rows, 8*H cols]
    btbd = np.zeros((128, 8 * H), np.float32)
    for j in range(8):
        btbd[j * ED:(j + 1) * ED, j * H:(j + 1) * H] = B.T
    wd["btbd"] = btbd.astype(BFNP).copy()
    wd["gl_rep"] = np.tile(g("g_att_l")[None, :], (128, 1)).astype(BFNP).copy()
    wd["gar_col"] = g("g_att_r")[:, None].copy()
    wd["gbias_col"] = g("g_bias")[:, None].copy()

    def gru_pack(pfx, wi, wh, bi, bh):
        bi = bi - wi.sum(1)
        wd[pfx + "wi_r"] = wi[0:H].T.copy()
        wd[pfx + "wh_r"] = wh[0:H].T.copy()
        wd[pfx + "wi_z"] = wi[H:2 * H].T.copy()
        wd[pfx + "wh_z"] = wh[H:2 * H].T.copy()
        wd[pfx + "wi_n"] = wi[2 * H:].T.copy()
        wd[pfx + "wh_n"] = wh[2 * H:].T.copy()
        wd[pfx + "br"] = (bi[0:H] + bh[0:H])[:, None].copy()
        wd[pfx + "bz"] = (bi[H:2 * H] + bh[H:2 * H])[:, None].copy()
        wd[pfx + "bin"] = bi[2 * H:][:, None].copy()
        wd[pfx + "bhn"] = bh[2 * H:][:, None].copy()

    gru_pack("g0_", g("gru0_wi"), g("gru0_wh"), g("gru0_bi"), g("gru0_bh"))
    for l in range(L - 1):
        wd[f"c{l}_rhs"] = g("conv_lin_w")[l].T.copy()           # [64,64] W.T
        wd[f"c{l}_asrep"] = np.tile(g("conv_att_src")[l][None, :], (128, 1)).astype(BFNP).copy()
        wd[f"c{l}_wad"] = (g("conv_lin_w")[l].T @ g("conv_att_dst")[l])[:, None].copy()
        wd[f"c{l}_bias"] = g("conv_bias")[l][:, None].copy()
        gru_pack(f"c{l}_", g("grul_wi")[l], g("grul_wh")[l],
                 g("grul_bi")[l], g("grul_bh")[l])
    wd["rhs_mol"] = np.concatenate([np.eye(H, dtype=np.float32),
                                    g("mol_lin_w").T], 1).copy()  # [64,128]
    wd["m_asrep"] = np.tile(g("mol_att_src")[None, :], (128, 1)).astype(BFNP).copy()
    wd["m_wad"] = (g("mol_lin_w").T @ g("mol_att_dst"))[:, None].copy()
    wd["m_bias"] = g("mol_bias")[:, None].copy()
    gru_pack("m_", g("mgru_wi"), g("mgru_wh"), g("mgru_bi"), g("mgru_bh"))
    wd["lin2_lhsT"] = g("lin2_w").T.copy()
    wd["lin2_bcol"] = g("lin2_b")[:, None].copy()
    wd["ones128"] = np.ones((1, 128), np.float32)
    return wd


# ======================================================================
def build_kernel(meta, wspecs):
    W_b, sumW, cumW = meta["W_b"], meta["sumW"], meta["cumW"]
    goff = meta["goff"]
    Gtot = meta["Gtot"]
    totslots = meta["totslots"]
    chunk_off = {(b, w0): co for (b, w0, cw, co) in meta["chunks"]}
    nc = bacc.Bacc(None, num_swdge_queues=4)

    def dp(n, s, dt=F32):
        return nc.declare_dram_parameter(n, list(s), dt, isOutput=False)

    xg_d = dp("xg16", [IN_C, NTAB], BF16)
    xo_d = dp("xo", [IN_C, NP])
    idx_d = dp("idx_all", [128, 8 * totslots], I16)
    maskf_d = dp("mask_f", [128, sumW])
    easl_d = dp("easl", [Gtot, 128, 128], BF16)
    molcol_d = dp("molcol", [128, NB])
    iota_d = dp("iotaF", [128, GMAX])
    w_d = {n: dp(n, s, dt) for n, (s, dt) in wspecs.items()}
    out_d = nc.declare_dram_parameter("out", [OUT_C, 128], F32, isOutput=True)
    dbg = {}
    if DEBUG:
        dbg["xA"] = nc.declare_dram_parameter("dbg_xA", [H, NP], F32, isOutput=True)
        dbg["hT"] = nc.declare_dram_parameter("dbg_hT", [H, NP], F32, isOutput=True)
        dbg["xB"] = nc.declare_dram_parameter("dbg_xB", [H, NP], F32, isOutput=True)
        dbg["t1"] = nc.declare_dram_parameter("dbg_t1", [1024, 128], BF16, isOutput=True)
        dbg["num"] = nc.declare_dram_parameter("dbg_num", [NB, 128, H + 1], F32, isOutput=True)
        dbg["at"] = nc.declare_dram_parameter("dbg_at", [NB, 128, WC], F32, isOutput=True)

    qn = [0]

    def nextq():
        if ONEQ:
            return 0
        qn[0] = (qn[0] + 1) % 4
        return qn[0]

    with tile.TileContext(nc) as tc:
        with tc.tile_pool(name="const", bufs=1) as cp, \
             tc.tile_pool(name="state", bufs=1) as st, \
             tc.tile_pool(name="wk5", bufs=6) as w5, \
             tc.tile_pool(name="wkio", bufs=3) as wio, \
             tc.tile_pool(name="ztp", bufs=3) as ztp, \
             tc.tile_pool(name="gbuf", bufs=3) as gp, \
             tc.tile_pool(name="small", bufs=2) as sm, \
             tc.tile_pool(name="gps", bufs=4, space="PSUM") as psg, \
             tc.tile_pool(name="vps", bufs=2, space="PSUM") as psv, \
             tc.tile_pool(name="mps", bufs=2, space="PSUM") as psm, \
             tc.tile_pool(name="dram", bufs=1, space="DRAM") as dr:

            wt = {}
            for n in wspecs:
                t = cp.tile(list(wspecs[n][0]), wspecs[n][1], tag=n)
                nc.sync.dma_start(t[:], w_d[n][:])
                wt[n] = t
            ident = cp.tile([128, 128], F32, tag="ident")
            make_identity(nc, ident[:])
            maskf = cp.tile([128, sumW], F32, tag="maskf")
            nc.sync.dma_start(maskf[:], maskf_d[:])
            idxt = cp.tile([128, 8 * totslots], I16, tag="idxt")
            nc.sync.dma_start(idxt[:], idx_d[:])
            molcol = cp.tile([128, NB], F32, tag="molcol")
            nc.sync.dma_start(molcol[:], molcol_d[:])
            iotaF = cp.tile([128, GMAX], F32, tag="iotaF")
            nc.sync.dma_start(iotaF[:], iota_d[:])
            rcol = cp.tile([128, NB], F32, tag="rcol")
            adcol2 = cp.tile([128, NB], F32, tag="adcol2")
            adcol3 = cp.tile([128, NB], F32, tag="adcol3")
            outT = cp.tile([H, 128], F32, tag="outT")

            xA = st.tile([H, NP], F32, tag="xA")
            xB = st.tile([H, NP], F32, tag="xB")
            hT = st.tile([H, NP], F32, tag="hT")
            xhb = st.tile([128, NB, 132], BF16, tag="xhb")
            smat = st.tile([128, NB, GMAX], BF16, tag="smat")
            asrcC = st.tile([128, NB], F32, tag="asrcC")

            table1 = dr.tile([NTAB, 128], BF16, tag="t1")
            tb_in = dr.tile([NP, 128], BF16, tag="tbin")
            table2 = dr.tile([NTAB, 128], BF16, tag="t2", addr_space="Shared")
            table3 = dr.tile([NTAB, 128], BF16, tag="t3", addr_space="Shared")

            def tbias(tbl):
                return _bap(tbl[:], [[128, NTAB - BIAS], [1, 128]], BIAS * 128)

            # -------- helpers --------
            def gather_chunk(buf, tab_ap, b, w0, cw):
                # gathers cw real slots + 1 trailing dummy slot (race pad)
                co = chunk_off[(b, w0)]
                nc.gpsimd.dma_gather(
                    out_ap=buf[:, 0:cw + 1, :], in_ap=tab_ap,
                    idxs_ap=idxt[:, 8 * co:8 * (co + cw + 1)],
                    num_idxs=128 * (cw + 1), num_idxs_reg=128 * (cw + 1),
                    elem_size=128, single_packet=False, queue_num=nextq())

            def softmax_chunk(asr, mask_ap, adcol_ap, cw):
                cnd = sm.tile([128, WC], F32, tag="cnd")
                nc.vector.tensor_scalar(out=cnd[:, :cw], in0=asr, scalar1=adcol_ap,
                                        scalar2=None, op0=OP.add)
                nc.scalar.activation(cnd[:, :cw], cnd[:, :cw], AF.Lrelu, alpha=SLOPE)
                nc.scalar.activation(cnd[:, :cw], cnd[:, :cw], AF.Exp)
                pm = sm.tile([128, WC], F32, tag="pm")
                nc.vector.tensor_tensor(out=pm[:, :cw], in0=cnd[:, :cw],
                                        in1=mask_ap, op=OP.mult)
                pmb = sm.tile([128, WC], BF16, tag="pmb")
                nc.vector.tensor_copy(pmb[:, :cw], pm[:, :cw])
                return pm, pmb

            def agg_chunk(msg_ap, pm, pmb, cw, num, zz, first):
                pmbb = _bap(pmb[:, 0:1], [pmb[:].ap[0], [1, cw], [0, H]])
                nc.vector.tensor_tensor(out=msg_ap, in0=msg_ap, in1=pmbb, op=OP.mult)
                mr = _bap(msg_ap, [msg_ap.ap[0], [1, H], [128, cw]])
                if first:
                    nc.vector.tensor_reduce(out=num[:], in_=mr,
                                            axis=mybir.AxisListType.X, op=OP.add)
                    nc.vector.tensor_reduce(out=zz[:], in_=pm[:, 0:cw],
                                            axis=mybir.AxisListType.X, op=OP.add)
                else:
                    part = sm.tile([128, H], F32, tag="part")
                    nc.vector.tensor_reduce(out=part[:], in_=mr,
                                            axis=mybir.AxisListType.X, op=OP.add)
                    nc.vector.tensor_tensor(out=num[:], in0=num[:], in1=part[:],
                                            op=OP.add)
                    zp = sm.tile([128, 1], F32, tag="zp")
                    nc.vector.tensor_reduce(out=zp[:], in_=pm[:, 0:cw],
                                            axis=mybir.AxisListType.X, op=OP.add)
                    nc.vector.tensor_tensor(out=zz[:], in0=zz[:], in1=zp[:],
                                            op=OP.add)

            def finish_block(num, zz, bias_col, b):
                nc.vector.tensor_scalar(out=zz[:], in0=zz[:], scalar1=1e-16,
                                        scalar2=None, op0=OP.add)
                rec = sm.tile([128, 1], F32, tag="rec")
                nc.vector.reciprocal(rec[:], zz[:])
                hnm = sm.tile([128, H], F32, tag="hnm")
                nc.vector.tensor_scalar(out=hnm[:], in0=num[:], scalar1=rec[:, 0:1],
                                        scalar2=None, op0=OP.mult)
                tps = psm.tile([128, 128], F32, tag="mps")
                nc.tensor.transpose(out=tps[0:H, :], in_=hnm[:], identity=ident[:])
                rp = sm.tile([H, 128], F32, tag="rp")
                nc.scalar.activation(rp[:], tps[0:H, :], AF.Relu, bias=bias_col)
                m0 = sm.tile([H, 128], F32, tag="m0")
                nc.vector.tensor_scalar(out=m0[:], in0=tps[0:H, :], scalar1=bias_col,
                                        scalar2=None, op0=OP.add)
                nc.vector.tensor_scalar(out=m0[:], in0=m0[:], scalar1=0.0,
                                        scalar2=None, op0=OP.min)
                nc.scalar.activation(m0[:], m0[:], AF.Exp)
                nc.vector.tensor_tensor(out=hT[:, b * 128:(b + 1) * 128],
                                        in0=rp[:], in1=m0[:], op=OP.add)

            def gru_chunk(pfx, h_full, x_in, x_out, c0, cw):
                sl = slice(c0, c0 + cw)
                h_ap = h_full[:, sl]
                x_ap = x_in[:, sl]
                rps = psg.tile([H, 512], F32, tag="gp")
                nc.tensor.matmul(rps[:, :cw], lhsT=wt[pfx + "wi_r"][:],
                                 rhs=h_ap, start=True, stop=False)
                nc.tensor.matmul(rps[:, :cw], lhsT=wt[pfx + "wh_r"][:],
                                 rhs=x_ap, start=False, stop=True)
                rsb = w5.tile([H, 512], F32, tag="w5")
                nc.scalar.activation(rsb[:, :cw], rps[:, :cw], AF.Sigmoid,
                                     bias=wt[pfx + "br"][:])
                zps = psg.tile([H, 512], F32, tag="gp")
                nc.tensor.matmul(zps[:, :cw], lhsT=wt[pfx + "wi_z"][:],
                                 rhs=h_ap, start=True, stop=False)
                nc.tensor.matmul(zps[:, :cw], lhsT=wt[pfx + "wh_z"][:],
                                 rhs=x_ap, start=False, stop=True)
                zsb = w5.tile([H, 512], F32, tag="w5")
                nc.scalar.activation(zsb[:, :cw], zps[:, :cw], AF.Sigmoid,
                                     bias=wt[pfx + "bz"][:])
                gin = psg.tile([H, 512], F32, tag="gp")
                nc.tensor.matmul(gin[:, :cw], lhsT=wt[pfx + "wi_n"][:],
                                 rhs=h_ap, start=True, stop=True)
                ghn = psg.tile([H, 512], F32, tag="gp")
                nc.tensor.matmul(ghn[:, :cw], lhsT=wt[pfx + "wh_n"][:],
                                 rhs=x_ap, start=True, stop=True)
                ghb = w5.tile([H, 512], F32, tag="w5")
                nc.scalar.activation(ghb[:, :cw], ghn[:, :cw], AF.Identity,
                                     bias=wt[pfx + "bhn"][:])
                rg = w5.tile([H, 512], F32, tag="w5")
                nc.vector.tensor_tensor(out=rg[:, :cw], in0=rsb[:, :cw],
                                        in1=ghb[:, :cw], op=OP.mult)
                nc.vector.tensor_tensor(out=rg[:, :cw], in0=rg[:, :cw],
                                        in1=gin[:, :cw], op=OP.add)
                nsb = w5.tile([H, 512], F32, tag="w5")
                nc.scalar.activation(nsb[:, :cw], rg[:, :cw], AF.Tanh,
                                     bias=wt[pfx + "bin"][:])
                dd = w5.tile([H, 512], F32, tag="w5")
                nc.vector.tensor_tensor(out=dd[:, :cw], in0=x_ap,
                                        in1=nsb[:, :cw], op=OP.subtract)
                nc.vector.tensor_tensor(out=dd[:, :cw], in0=dd[:, :cw],
                                        in1=zsb[:, :cw], op=OP.mult)
                nc.vector.tensor_tensor(out=dd[:, :cw], in0=dd[:, :cw],
                                        in1=nsb[:, :cw], op=OP.add)
                nc.scalar.activation(x_out[:, sl], dd[:, :cw], AF.Relu)

            # PE-transpose an H-major f32 [p, 128] slice into bf16 [128, p]
            def pe_transpose_store(src_ap, p, dst_ap):
                tp = psm.tile([128, 128], F32, tag="mps")
                nc.tensor.transpose(out=tp[:, 0:p], in_=src_ap,
                                    identity=ident[0:p, 0:p])
                tt = sm.tile([128, 128], BF16, tag="tt")
                nc.vector.tensor_copy(tt[:, 0:p], tp[:, 0:p])
                nc.sync.dma_start(dst_ap, tt[:, 0:p])

            # readout prep per 128-block (from x4 = final x state)
            def readout_prep_block(x_src, b):
                xp = psm.tile([128, 128], F32, tag="mps")
                nc.tensor.matmul(xp[:], lhsT=x_src[:, b * 128:(b + 1) * 128],
                                 rhs=wt["rhs_mol"][:], start=True, stop=True)
                nc.vector.tensor_copy(xhb[:, b, 0:128], xp[:])
                nc.vector.memset(xhb[:, b, 128:129], 1.0)
                # S one-hot
                mc = _bap(molcol[:, b:b + 1], [molcol[:].ap[0], [0, GMAX]])
                nc.vector.tensor_tensor(out=smat[:, b, :], in0=mc, in1=iotaF[:],
                                        op=OP.is_equal)
                # asrc = hs . att_src
                zt = ztp.tile([128, 8, H], BF16, tag="zt")
                nc.vector.tensor_tensor(out=zt[:, 0, :], in0=xhb[:, b, 64:128],
                                        in1=wt["m_asrep"][:, 0:H], op=OP.mult)
                nc.vector.tensor_reduce(out=asrcC[:, b:b + 1], in_=zt[:, 0, :],
                                        axis=mybir.AxisListType.X, op=OP.add)

            # ---------- stage 0: table1 (redundant on all cores) ----------
            for c in range(NTAB // 512):
                xc = wio.tile([IN_C, 512], BF16, tag="xg")
                nc.sync.dma_start(xc[:], xg_d[:, c * 512:(c + 1) * 512])
                x1p = psg.tile([H, 512], F32, tag="gp")
                nc.tensor.matmul(x1p[:], lhsT=wt["lin1_lhsT16"][:], rhs=xc[:],
                                 start=True, stop=True)
                x1s = wio.tile([H, 512], BF16, tag="x1s")
                nc.scalar.activation(x1s[:], x1p[:], AF.Lrelu,
                                     bias=wt["b1col"][:], alpha=SLOPE)
                t1p = psv.tile([128, 512], F32, tag="vps")
                nc.tensor.matmul(t1p[:], lhsT=wt["m1_lhsT"][:], rhs=x1s[:],
                                 start=True, stop=True)
                t1s = wio.tile([128, 512], F32, tag="t1s")
                nc.vector.tensor_copy(t1s[:], t1p[:])
                for s in range(4):
                    pe_transpose_store(
                        t1s[:, s * 128:(s + 1) * 128], 128,
                        table1[c * 512 + s * 128:c * 512 + (s + 1) * 128, :])

            c0 = 0
            while c0 < NP:
                cw = min(512, NP - c0)
                xc = wio.tile([IN_C, 512], F32, tag="xo")
                nc.sync.dma_start(xc[:, :cw], xo_d[:, c0:c0 + cw])
                x1p = psg.tile([H, 512], F32, tag="gp")
                nc.tensor.matmul(x1p[:, :cw], lhsT=wt["lin1_lhsT"][:], rhs=xc[:, :cw],
                                 start=True, stop=True)
                nc.scalar.activation(xA[:, c0:c0 + cw], x1p[:, :cw], AF.Lrelu,
                                     bias=wt["b1col"][:], alpha=SLOPE)
                c0 += cw
            for b in range(NB):
                rp_ = psm.tile([128, 128], F32, tag="mps")
                nc.tensor.matmul(rp_[:, 0:1], lhsT=xA[:, b * 128:(b + 1) * 128],
                                 rhs=wt["gar_col"][:], start=True, stop=True)
                nc.vector.tensor_copy(rcol[:, b:b + 1], rp_[:, 0:1])

            # ---------- conv layers ----------
            def conv_layer(lidx, x_in, x_out, tab, adcol_t, post):
                """lidx: -1 = GATEConv, 0/1 = GATConv; post(b_done) emitted
                after finish_block when a 512-chunk of hT is complete."""
                tb = tbias(tab)
                if lidx == -1:
                    gpfx, bias_col, acol = "g0_", wt["gbias_col"][:], rcol
                else:
                    gpfx = f"c{lidx}_"
                    bias_col = wt[f"c{lidx}_bias"][:]
                    acol = adcol_t
                done = [0]
                for b in range(NB):
                    w = W_b[b]
                    num = sm.tile([128, H], F32, tag="num")
                    zz = sm.tile([128, 1], F32, tag="zz")
                    w0 = 0
                    first = True
                    while w0 < w:
                        cw = min(WC, w - w0)
                        bufE = gp.tile([128, WC + 1, 128], BF16, tag="gbuf")
                        gather_chunk(bufE, tb, b, w0, cw)
                        at = sm.tile([128, WC], F32, tag="at")
                        for g0 in range(0, cw, 8):
                            gw = min(8, cw - g0)
                            zt = ztp.tile([128, 8, H], BF16, tag="zt")
                            if lidx == -1:
                                eat = wio.tile([128, 128], BF16, tag="eat")
                                gi = goff[b] + (w0 + g0) // 8
                                nc.sync.dma_start(eat[:], easl_d[gi])
                                vps = psv.tile([128, 512], F32, tag="vps")
                                nc.tensor.matmul(vps[:, :gw * H], lhsT=eat[:],
                                                 rhs=wt["btbd"][:, :gw * H],
                                                 start=True, stop=True)
                                eb = ztp.tile([128, 8, H], BF16, tag="eb")
                                vps3 = _bap(vps[:], [vps[:].ap[0], [H, gw], [1, H]])
                                nc.scalar.activation(eb[:, :gw, :], vps3,
                                                     AF.Identity)
                                nc.vector.tensor_tensor(
                                    out=zt[:, :gw, :],
                                    in0=bufE[:, g0:g0 + gw, 0:H],
                                    in1=eb[:, :gw, :], op=OP.add)
                                nc.scalar.activation(zt[:, :gw, :], zt[:, :gw, :],
                                                     AF.Lrelu, alpha=SLOPE)
                                glb = _bap(wt["gl_rep"][:],
                                           [wt["gl_rep"][:].ap[0], [0, gw], [1, H]])
                                nc.vector.tensor_tensor(out=zt[:, :gw, :],
                                                        in0=zt[:, :gw, :],
                                                        in1=glb, op=OP.mult)
                            else:
                                asb = _bap(wt[f"c{lidx}_asrep"][:],
                                           [wt[f"c{lidx}_asrep"][:].ap[0],
                                            [0, gw], [1, H]])
                                nc.vector.tensor_tensor(
                                    out=zt[:, :gw, :],
                                    in0=bufE[:, g0:g0 + gw, 0:H],
                                    in1=asb, op=OP.mult)
                            nc.vector.tensor_reduce(out=at[:, g0:g0 + gw],
                                                    in_=zt[:, :gw, :],
                                                    axis=mybir.AxisListType.X,
                                                    op=OP.add)
                        if DEBUG and lidx == -1 and w0 == 0:
                            nc.sync.dma_start(dbg["at"][b, :, 0:cw], at[:, 0:cw])
                        pm, pmb = softmax_chunk(
                            at[:, 0:cw], maskf[:, cumW[b] + w0:cumW[b] + w0 + cw],
                            acol[:, b:b + 1], cw)
                        msg = bufE[:, 0:cw, (H if lidx == -1 else 0):
                                   (2 * H if lidx == -1 else H)]
                        agg_chunk(msg, pm, pmb, cw, num, zz, first)
                        first = False
                        w0 += cw
                    if DEBUG and lidx == -1:
                        nc.sync.dma_start(dbg["num"][b, :, 0:H], num[:])
                        nc.sync.dma_start(dbg["num"][b, :, H:H + 1], zz[:])
                    finish_block(num, zz, bias_col, b)
                    if (b + 1) % 4 == 0 or b == NB - 1:
                        c0 = done[0]
                        cw = 128 * (b + 1) - c0
                        while cw > 0:
                            cc = min(512, cw)
                            gru_chunk(gpfx, hT, x_in, x_out, c0, cc)
                            post(c0, cc)
                            c0 += cc
                            cw -= cc
                        done[0] = 128 * (b + 1)

            # L1: GATEConv; builds table2 rows from xB
            def post1(c0, cw):
                hsp = psg.tile([H, 512], F32, tag="gp")
                nc.tensor.matmul(hsp[:, :cw], lhsT=wt["c0_rhs"][:],
                                 rhs=xB[:, c0:c0 + cw], start=True, stop=True)
                hsb = wio.tile([H, 512], F32, tag="hsb16")
                nc.vector.tensor_copy(hsb[:, :cw], hsp[:, :cw])
                for s in range(0, cw, 128):
                    pe_transpose_store(
                        hsb[:, s:s + 128], H,
                        _bap(tb_in[:], [[128, 128], [1, H]], (c0 + s) * 128))
                    b = (c0 + s) // 128
                    adp = psm.tile([128, 128], F32, tag="mps")
                    nc.tensor.matmul(adp[:, 0:1],
                                     lhsT=xB[:, c0 + s:c0 + s + 128],
                                     rhs=wt["c0_wad"][:], start=True, stop=True)
                    nc.vector.tensor_copy(adcol2[:, b:b + 1], adp[:, 0:1])
            conv_layer(-1, xA, xB, table1, None, post1)
            if DEBUG:
                nc.sync.dma_start(dbg["xA"][:], xA[:])
                nc.sync.dma_start(dbg["hT"][:], hT[:])
                nc.sync.dma_start(dbg["xB"][:], xB[:])
                nc.sync.dma_start(dbg["t1"][:], table1[0:1024, :])
            nc.gpsimd.collective_compute(
                "AllGather", OP.bypass, replica_groups=[list(range(NDEV))],
                ins=[tb_in[:]], outs=[table2[:]])

            # L2: GATConv 0; builds table3 rows from xA(out)
            def post2(c0, cw):
                hsp = psg.tile([H, 512], F32, tag="gp")
                nc.tensor.matmul(hsp[:, :cw], lhsT=wt["c1_rhs"][:],
                                 rhs=xA[:, c0:c0 + cw], start=True, stop=True)
                hsb = wio.tile([H, 512], F32, tag="hsb16")
                nc.vector.tensor_copy(hsb[:, :cw], hsp[:, :cw])
                for s in range(0, cw, 128):
                    pe_transpose_store(
                        hsb[:, s:s + 128], H,
                        _bap(tb_in[:], [[128, 128], [1, H]], (c0 + s) * 128))
                    b = (c0 + s) // 128
                    adp = psm.tile([128, 128], F32, tag="mps")
                    nc.tensor.matmul(adp[:, 0:1],
                                     lhsT=xA[:, c0 + s:c0 + s + 128],
                                     rhs=wt["c1_wad"][:], start=True, stop=True)
                    nc.vector.tensor_copy(adcol3[:, b:b + 1], adp[:, 0:1])
            conv_layer(0, xB, xA, table2, adcol2, post2)
            nc.gpsimd.collective_compute(
                "AllGather", OP.bypass, replica_groups=[list(range(NDEV))],
                ins=[tb_in[:]], outs=[table3[:]])

            # L3: GATConv 1; readout prep from xB(out)
            def post3(c0, cw):
                for s in range(0, cw, 128):
                    readout_prep_block(xB, (c0 + s) // 128)
            conv_layer(1, xA, xB, table3, adcol3, post3)

            # ---------- molecule readout (matmul-based) ----------
            # pass 0: out = relu(sum_x per mol)
            sump = psm.tile([128, 128], F32, tag="mps")
            for b in range(NB):
                nc.tensor.matmul(sump[0:GMAX, 0:H], lhsT=smat[:, b, :],
                                 rhs=xhb[:, b, 0:H],
                                 start=(b == 0), stop=(b == NB - 1))
            sums = sm.tile([GMAX, H], F32, tag="sums")
            nc.vector.tensor_copy(sums[:], sump[0:GMAX, 0:H])
            o0ps = psv.tile([128, 512], F32, tag="vps")
            nc.tensor.transpose(out=o0ps[0:H, 0:GMAX],
                                in_=sums[:], identity=ident[0:GMAX, 0:GMAX])
            nc.vector.memset(outT[:], 0.0)
            nc.scalar.activation(outT[:, 0:GMAX], o0ps[0:H, 0:GMAX], AF.Relu)

            hm = cp.tile([H, 128], F32, tag="hm")
            gout = cp.tile([H, 128], F32, tag="gout")
            ndf = sm.tile([GMAX, 132], F32, tag="ndf")
            for t in range(T):
                adp = psm.tile([128, 128], F32, tag="mps")
                nc.tensor.matmul(adp[0:1, 0:GMAX], lhsT=wt["m_wad"][:],
                                 rhs=outT[:, 0:GMAX], start=True, stop=True)
                adr = sm.tile([1, GMAX], F32, tag="adr")
                nc.vector.tensor_copy(adr[:], adp[0:1, 0:GMAX])
                adBp = psv.tile([128, 512], F32, tag="vps")
                nc.tensor.matmul(adBp[:, 0:GMAX], lhsT=wt["ones128"][:],
                                 rhs=adr[:], start=True, stop=True)
                adB = sm.tile([128, GMAX], BF16, tag="adB")
                nc.vector.tensor_copy(adB[:], adBp[:, 0:GMAX])
                ndp = psm.tile([128, 128], F32, tag="mps")
                for b in range(NB):
                    zt = ztp.tile([128, 8, H], BF16, tag="zt")
                    nc.vector.tensor_tensor(out=_bap(zt[:], [zt[:].ap[0], [1, GMAX]]),
                                            in0=smat[:, b, :], in1=adB[:],
                                            op=OP.mult)
                    adsel = sm.tile([128, 1], F32, tag="adsel")
                    nc.vector.tensor_reduce(
                        out=adsel[:],
                        in_=_bap(zt[:], [zt[:].ap[0], [1, GMAX]]),
                        axis=mybir.AxisListType.X, op=OP.add)
                    acol = sm.tile([128, 1], F32, tag="acol")
                    nc.vector.tensor_tensor(out=acol[:], in0=asrcC[:, b:b + 1],
                                            in1=adsel[:], op=OP.add)
                    nc.scalar.activation(acol[:], acol[:], AF.Lrelu, alpha=SLOPE)
                    nc.scalar.activation(acol[:], acol[:], AF.Exp)
                    pcol = sm.tile([128, 1], BF16, tag="pcol")
                    nc.vector.tensor_copy(pcol[:], acol[:])
                    spb = sm.tile([128, GMAX], BF16, tag="spb")
                    pb = _bap(pcol[:, 0:1], [pcol[:].ap[0], [0, GMAX]])
                    nc.vector.tensor_tensor(out=spb[:], in0=smat[:, b, :],
                                            in1=pb, op=OP.mult)
                    nc.tensor.matmul(ndp[0:GMAX, 0:H + 1], lhsT=spb[:],
                                     rhs=xhb[:, b, 64:129],
                                     start=(b == 0), stop=(b == NB - 1))
                nc.vector.tensor_copy(ndf[:, 0:H + 1], ndp[0:GMAX, 0:H + 1])
                nc.vector.tensor_scalar(out=ndf[:, H:H + 1], in0=ndf[:, H:H + 1],
                                        scalar1=1e-16, scalar2=None, op0=OP.add)
                rec = sm.tile([GMAX, 1], F32, tag="recm")
                nc.vector.reciprocal(rec[:], ndf[:, H:H + 1])
                hnm = sm.tile([GMAX, H], F32, tag="hnmm")
                nc.vector.tensor_scalar(out=hnm[:], in0=ndf[:, 0:H],
                                        scalar1=rec[:, 0:1],
                                        scalar2=None, op0=OP.mult)
                tps = psv.tile([128, 512], F32, tag="vps")
                nc.tensor.transpose(out=tps[0:H, 0:GMAX], in_=hnm[:],
                                    identity=ident[0:GMAX, 0:GMAX])
                rp = sm.tile([H, GMAX], F32, tag="rpm")
                nc.scalar.activation(rp[:], tps[0:H, 0:GMAX], AF.Relu,
                                     bias=wt["m_bias"][:])
                m0 = sm.tile([H, GMAX], F32, tag="m0m")
                nc.vector.tensor_scalar(out=m0[:], in0=tps[0:H, 0:GMAX],
                                        scalar1=wt["m_bias"][:],
                                        scalar2=None, op0=OP.add)
                nc.vector.tensor_scalar(out=m0[:], in0=m0[:], scalar1=0.0,
                                        scalar2=None, op0=OP.min)
                nc.scalar.activation(m0[:], m0[:], AF.Exp)
                nc.vector.memset(hm[:], 0.0)
                nc.vector.tensor_tensor(out=hm[:, 0:GMAX], in0=rp[:], in1=m0[:],
                                        op=OP.add)
                gru_chunk("m_", hm[:], outT[:], gout[:], 0, 128)
                nc.vector.tensor_copy(outT[:], gout[:])

            resp = psm.tile([128, 128], F32, tag="mps")
            nc.tensor.matmul(resp[:], lhsT=wt["lin2_lhsT"][:], rhs=outT[:],
                             start=True, stop=True)
            rsb = sm.tile([OUT_C, 128], F32, tag="rsb2")
            nc.scalar.activation(rsb[:], resp[:], AF.Identity, bias=wt["lin2_bcol"][:])
            nc.sync.dma_start(out_d[:], rsb[:])

    nc.compile()
    return nc


# ======================================================================
def kernel(**inputs):
    meta, arrs = preprocess(inputs)
    wd = pack_weights(inputs)
    wspecs = {}
    for n, v in wd.items():
        dt = BF16 if v.dtype == BFNP else F32
        wspecs[n] = (v.shape, dt)
    t0 = time.time()
    nc = build_kernel(meta, wspecs)
    print(f"[kernel] build+compile {time.time()-t0:.1f}s", file=sys.stderr)

    in_maps = []
    for k in range(NDEV):
        m = dict(xg16=arrs["xg16"], xo=arrs["xo"][k],
                 idx_all=arrs["idx_all"][k], mask_f=arrs["mask_f"][k],
                 easl=arrs["easl"][k], molcol=arrs["molcol"][k],
                 iotaF=arrs["iotaF"])
        m.update(wd)
        in_maps.append(m)

    res = run_bass_kernel_spmd(nc, in_maps, list(range(NDEV)), trace=TRACE)
    LAST_EXEC_NS[0] = res.exec_time_ns
    LAST_RESULTS[0] = res.results

    out = np.zeros((G, OUT_C), np.float32)
    gcuts = meta["gcuts"]
    for k in range(NDEV):
        gk = meta["gkc"][k]
        out[gcuts[k]:gcuts[k] + gk] = res.results[k]["out"][:, :gk].T
    return np.repeat(out[None], OUT_N, axis=0)
